# revision 11
# baseline (speedup 1.0000x reference)
"""Trainium2 Bass kernel for nn_BinaryDense: out = x @ (sum_k sign(b_k)*a_k) + bias.

Shapes (hardcoded): x [4096,4096] f32, b [4,4096,4096] f32, a [4,4096] f32,
bias [4096] f32 -> out [4096,4096] f32.

Tensor-parallel over the output (units) dim across 8 NeuronCores; core c owns
O-columns [c*512, (c+1)*512).

Per core: one bf16 matmul x @ w (lhsT = host-pretransposed x^T tiles
stationary, w tiles moving, fp32 PSUM) with the weight w built on-chip:
w[:, oc] = sum_k copysign(a[k,oc], b[k,:,oc]).

v2 design (from ntff trace analysis of the 264.6us baseline):
  - b tiles stream on the Scalar-engine HWDGE ring, x^T tiles on the Sync
    ring: two independent hardware DMA queues, so the startup w-build is
    never starved behind x traffic.
  - w-build on DVE is one fused scalar_tensor_tensor ((b & 0x8000) | a,
    int16 lanes) + two bf16 pair-adds for the k-sum.
  - HAM warm-up: ~11 dummy matmuls issued during the DMA/build latency so
    the PE clock-gate opens before the first real matmul; a few filler
    matmuls inside the first m-block keep it open across build-chase stalls.
  - K-blocks [4,8,20]: the kb0/kb1 partial sums are evicted by the Scalar
    engine (PSUM->SBUF copy, bf16), combined + biased on GpSimd (SBUF only),
    so DVE does nothing but builds until the single final add per m-tile
    (out = psum + acc) in kb2. No DVE evict backlog -> no PE psum stalls.
  - Last m-block runs j-outer (per-m-tile complete groups) so the final
    adds + out-DMAs of 3 of its 4 m-tiles overlap the remaining matmuls
    (short tail). Out tiles leave on the Scalar ring (idle after b).

Host side only reshapes/casts/shards (no math): x^T bf16, b -> [I,K,O] bf16,
a/bias broadcast rows (bias bf16: adds ~0.3% of one bf16 ulp vs |out|~74).
"""

import os
import sys

if "/opt/trn_rl_repo" not in sys.path:
    sys.path.insert(0, "/opt/trn_rl_repo")

import numpy as np
import ml_dtypes

BF16 = ml_dtypes.bfloat16

B = 4096   # batch rows of x
I = 4096   # input dim (contraction)
O = 4096   # output dim (sharded)
K = 4      # binary bases
NCORES = 8
OC = O // NCORES   # 512 output cols per core
P = 128

KT = I // P        # 32 k-tiles (contraction)
MT = B // P        # 32 m-tiles (output rows)
M_BLOCK = 4        # m-tiles per psum block (4 banks, x2 parity = 8)
NMB = MT // M_BLOCK


def _build_program():
    import concourse.bass as bass
    import concourse.mybir as mybir
    from concourse import bacc
    from concourse.tile import TileContext

    K_BLOCKS = [int(s) for s in os.environ.get("BK_KBLOCKS", "4,10,18").split(",")]
    assert sum(K_BLOCKS) == KT
    N_DUM = int(os.environ.get("BK_DUMMIES", "14"))
    N_FILL = int(os.environ.get("BK_FILL", "8"))
    FUSED = os.environ.get("BK_FUSED", "0") == "1"
    UPFRONT_B = int(os.environ.get("BK_UPFRONT_B", "8"))
    DB_PER_UNIT = int(os.environ.get("BK_DB_PER_UNIT", "3"))
    ADD1_GPS_EVERY = int(os.environ.get("BK_ADD1_GPS_EVERY", "2"))

    nc = bacc.Bacc(None, target_bir_lowering=False)

    b_re = nc.declare_dram_parameter("b_re", [I, K * OC], mybir.dt.bfloat16, isOutput=False)
    a_b = nc.declare_dram_parameter("a_b", [P, K * OC], mybir.dt.bfloat16, isOutput=False)
    xT = nc.declare_dram_parameter("xT", [I, B], mybir.dt.bfloat16, isOutput=False)
    bias_b = nc.declare_dram_parameter("bias_b", [P, OC], mybir.dt.bfloat16, isOutput=False)
    out = nc.declare_dram_parameter("out", [B, OC], mybir.dt.float32, isOutput=True)

    # unit schedule: one unit per (k-block, m-block)
    units = []
    k0 = 0
    for kb, KB in enumerate(K_BLOCKS):
        kts = list(range(k0, k0 + KB))
        for mb in range(NMB):
            units.append((kb, mb, kts))
        k0 += KB
    NKB = len(K_BLOCKS)

    with TileContext(nc) as tc:
        with (
            tc.tile_pool(name="const", bufs=1) as const,
            tc.tile_pool(name="bpool", bufs=7) as bpool,
            tc.tile_pool(name="cpool", bufs=3) as cpool,
            tc.tile_pool(name="tpool", bufs=3) as tpool,
            tc.tile_pool(name="wpool", bufs=1) as wpool,
            tc.tile_pool(name="xpool", bufs=24) as xpool,
            tc.tile_pool(name="accA", bufs=1) as apool,
            tc.tile_pool(name="accB", bufs=1) as apool2,
            tc.tile_pool(name="opool", bufs=4) as opool,
            tc.tile_pool(name="psum", bufs=1, space="PSUM") as psum_pool,
        ):
            # ---- consts ----
            a_tile = const.tile([P, K * OC], mybir.dt.bfloat16)
            nc.sync.dma_start(out=a_tile[:], in_=a_b[:, :])
            bias_tile = const.tile([P, OC], mybir.dt.bfloat16)
            nc.gpsimd.dma_start(out=bias_tile[:], in_=bias_b[:, :])
            mask16 = const.tile([P, 1], mybir.dt.int16)
            nc.vector.memset(mask16[:], -32768)       # 0x8000
            mask32 = const.tile([P, 1], mybir.dt.int32)
            nc.vector.memset(mask32[:], -2147450880)  # 0x80008000
            dummy_w = const.tile([P, P], mybir.dt.bfloat16)
            nc.vector.memset(dummy_w[:], 0)
            dummy_rhs = const.tile([P, OC], mybir.dt.bfloat16)
            nc.vector.memset(dummy_rhs[:], 0)

            b_tiles, w_tiles, xt_tiles = {}, {}, {}
            acc_a = {m: apool.tile([P, OC], mybir.dt.bfloat16, name=f"acc_a_{m}")
                     for m in range(MT)}
            acc_b = {m: apool2.tile([P, OC], mybir.dt.bfloat16, name=f"acc_b_{m}")
                     for m in range(MT)}

            def emit_bdma(kt):
                # NOTE: the Tile framework derives dependencies from emission
                # order — the doorbell MUST be emitted before the build that
                # reads the tile.
                bt = bpool.tile([P, K * OC], mybir.dt.bfloat16, name="b_tile")
                nc.scalar.dma_start(out=bt[:], in_=b_re[kt * P:(kt + 1) * P, :])
                b_tiles[kt] = bt

            def emit_build(kt):
                bt = b_tiles.pop(kt)
                c = cpool.tile([P, K * OC], mybir.dt.bfloat16, name="contrib")
                if FUSED:
                    nc.vector.scalar_tensor_tensor(
                        out=c.bitcast(mybir.dt.int16)[:],
                        in0=bt.bitcast(mybir.dt.int16)[:],
                        scalar=mask16[:, 0:1],
                        in1=a_tile.bitcast(mybir.dt.int16)[:],
                        op0=mybir.AluOpType.bitwise_and,
                        op1=mybir.AluOpType.bitwise_or,
                    )
                else:
                    nc.vector.tensor_scalar(
                        out=bt.bitcast(mybir.dt.int32)[:],
                        in0=bt.bitcast(mybir.dt.int32)[:],
                        scalar1=mask32[:, 0:1], scalar2=None,
                        op0=mybir.AluOpType.bitwise_and,
                    )
                    nc.vector.tensor_tensor(
                        out=c.bitcast(mybir.dt.int16)[:],
                        in0=bt.bitcast(mybir.dt.int16)[:],
                        in1=a_tile.bitcast(mybir.dt.int16)[:],
                        op=mybir.AluOpType.bitwise_or,
                    )
                t = tpool.tile([P, 2 * OC], mybir.dt.bfloat16, name="t_tile")
                # gpsimd add1-assist only for later tiles: gpsimd tt is slow
                # (~3.4us) so it must stay off the early critical chain, but
                # every assisted tile removes ~0.7us from the DVE build stream.
                add1_eng = (nc.gpsimd
                            if (ADD1_GPS_EVERY and kt >= 5
                                and kt % ADD1_GPS_EVERY == 1)
                            else nc.vector)
                add1_eng.tensor_tensor(
                    out=t[:], in0=c[:, 0:2 * OC], in1=c[:, 2 * OC:4 * OC],
                    op=mybir.AluOpType.add)
                w = wpool.tile([P, OC], mybir.dt.bfloat16, name=f"w_{kt}")
                nc.vector.tensor_tensor(
                    out=w[:], in0=t[:, 0:OC], in1=t[:, OC:2 * OC],
                    op=mybir.AluOpType.add)
                w_tiles[kt] = w

            def emit_xt(kt, mb):
                xt = xpool.tile([P, M_BLOCK * P], mybir.dt.bfloat16, name="xt")
                nc.sync.dma_start(
                    out=xt[:],
                    in_=xT[kt * P:(kt + 1) * P, mb * M_BLOCK * P:(mb + 1) * M_BLOCK * P])
                xt_tiles[(kt, mb)] = xt

            fill_parity = [0]

            def emit_filler(n):
                # alternate two banks so back-to-back dummy groups don't
                # serialize on same-bank pipeline drains
                for _ in range(n):
                    dps = psum_pool.tile([P, OC], mybir.dt.float32,
                                         name=f"ps_{4 + fill_parity[0]}")
                    fill_parity[0] ^= 1
                    nc.tensor.matmul(dps[:], dummy_w[:], dummy_rhs[:],
                                     start=True, stop=True)

            # ---- upfront: b doorbells, xt prefetch, HAM warm-up, early builds
            for kt in range(UPFRONT_B):
                emit_bdma(kt)
            for u in (0, 1):
                _, mb, kts = units[u]
                for kt in kts:
                    emit_xt(kt, mb)
            emit_filler(N_DUM)
            for kt in range(UPFRONT_B):
                emit_build(kt)
            bnext = UPFRONT_B

            # ---- main loop ----
            for u, (kb, mb, kts) in enumerate(units):
                parity = u % 2
                ps = {j: psum_pool.tile([P, OC], mybir.dt.float32,
                                        name=f"ps_{parity * 4 + j}")
                      for j in range(M_BLOCK)}
                nxt = units[u + 1] if u + 1 < len(units) else None
                last_unit = u == len(units) - 1

                if last_unit:
                    # j-outer: complete each m-tile's group, evict + store it
                    # while the next m-tile's matmuls still stream.
                    for j in range(M_BLOCK):
                        m = mb * M_BLOCK + j
                        for kt in kts:
                            nc.tensor.matmul(
                                ps[j][:],
                                xt_tiles[(kt, mb)][:, j * P:(j + 1) * P],
                                w_tiles[kt][:],
                                start=(kt == kts[0]), stop=(kt == kts[-1]))
                        o = opool.tile([P, OC], mybir.dt.float32, name="o_tile")
                        nc.vector.tensor_tensor(
                            out=o[:], in0=ps[j][:], in1=acc_a[m][:],
                            op=mybir.AluOpType.add)
                        nc.scalar.dma_start(out=out[m * P:(m + 1) * P, :], in_=o[:])
                    for kt in kts:
                        xt_tiles.pop((kt, mb))
                else:
                    done_pf = 0
                    for i, kt in enumerate(kts):
                        xt = xt_tiles.pop((kt, mb))
                        for j in range(M_BLOCK):
                            nc.tensor.matmul(
                                ps[j][:], xt[:, j * P:(j + 1) * P], w_tiles[kt][:],
                                start=(kt == kts[0]), stop=(kt == kts[-1]))
                        if u == 0 and i < len(kts) - 1:
                            emit_filler(N_FILL)
                        if u >= 1 and nxt is not None:
                            nkts = nxt[2]
                            tgt = min(((i + 1) * len(nkts) + len(kts) - 1) // len(kts),
                                      len(nkts))
                            while done_pf < tgt:
                                emit_xt(nkts[done_pf], nxt[1])
                                done_pf += 1

                    # evicts
                    for j in range(M_BLOCK):
                        m = mb * M_BLOCK + j
                        if kb == 0:
                            nc.scalar.copy(out=acc_a[m][:], in_=ps[j][:])
                        elif kb < NKB - 1:
                            nc.scalar.copy(out=acc_b[m][:], in_=ps[j][:])
                        else:
                            o = opool.tile([P, OC], mybir.dt.float32, name="o_tile")
                            nc.vector.tensor_tensor(
                                out=o[:], in0=ps[j][:], in1=acc_a[m][:],
                                op=mybir.AluOpType.add)
                            nc.scalar.dma_start(out=out[m * P:(m + 1) * P, :], in_=o[:])
                # paced b doorbells + builds (build pace must match the
                # HW ring's ~2.4us/tile, not throttle it)
                while bnext <= min(UPFRONT_B - 1 + DB_PER_UNIT * (u + 1), KT - 1):
                    emit_bdma(bnext)
                    emit_build(bnext)
                    bnext += 1

                if (not last_unit) and kb == NKB - 2:
                    # acc_a[m] += acc_b[m]; acc_a[m] += bias. bf16 SBUF adds
                    # are cheap on DVE (2x_1P, ~0.33us); emitted after this
                    # unit's build quota so they don't delay the build stream.
                    for j in range(M_BLOCK):
                        m = mb * M_BLOCK + j
                        nc.vector.tensor_tensor(
                            out=acc_a[m][:], in0=acc_a[m][:], in1=acc_b[m][:],
                            op=mybir.AluOpType.add)
                        nc.vector.tensor_tensor(
                            out=acc_a[m][:], in0=acc_a[m][:], in1=bias_tile[:],
                            op=mybir.AluOpType.add)

    nc.compile()
    return nc


_NC_CACHE = None


def _get_program():
    global _NC_CACHE
    if _NC_CACHE is None:
        _NC_CACHE = _build_program()
    return _NC_CACHE


def prep_inputs(x, b, a, bias):
    """Host-side shard/cast/layout only. Returns per-core input maps."""
    x = np.asarray(x, dtype=np.float32)
    b = np.asarray(b, dtype=np.float32)
    a = np.asarray(a, dtype=np.float32)
    bias = np.asarray(bias, dtype=np.float32)
    xT16 = np.ascontiguousarray(x.T).astype(BF16)          # [I, B] bf16
    b_iko = np.transpose(b, (1, 0, 2)).astype(BF16)        # [I, K, O] bf16
    a16 = a.astype(BF16)                                    # [K, O]
    bias16 = bias.astype(BF16)

    in_maps = []
    for c in range(NCORES):
        sl = slice(c * OC, (c + 1) * OC)
        b_slice = np.ascontiguousarray(b_iko[:, :, sl]).reshape(I, K * OC)
        a_flat = np.ascontiguousarray(a16[:, sl]).reshape(1, K * OC)
        a_bcast = np.broadcast_to(a_flat, (P, K * OC)).copy()
        bias_bcast = np.broadcast_to(bias16[sl].reshape(1, OC), (P, OC)).copy()
        in_maps.append({
            "b_re": b_slice,
            "a_b": a_bcast,
            "xT": xT16,
            "bias_b": bias_bcast,
        })
    return in_maps


def run(in_maps, trace=False):
    from concourse.bass_utils import run_bass_kernel_spmd

    nc = _get_program()
    res = run_bass_kernel_spmd(nc, in_maps, list(range(NCORES)), trace=trace)
    return res


def kernel(x, b, a, bias):
    in_maps = prep_inputs(x, b, a, bias)
    res = run(in_maps)
    out = np.concatenate([res.results[c]["out"] for c in range(NCORES)], axis=1)
    return np.ascontiguousarray(out, dtype=np.float32)


if __name__ == "__main__":
    rng = np.random.default_rng(0)
    x = rng.standard_normal((B, I), dtype=np.float32)
    b = rng.standard_normal((K, I, O), dtype=np.float32)
    a = rng.random((K, O), dtype=np.float32)
    bias = rng.standard_normal(O, dtype=np.float32)
    out = kernel(x=x, b=b, a=a, bias=bias)
    w_eff = np.einsum('kio,ko->io', np.sign(b), a.astype(np.float64)).astype(np.float64)
    expected = x.astype(np.float64) @ w_eff + bias
    rel = np.linalg.norm(out - expected) / np.linalg.norm(expected)
    print(f"rel_err = {rel:.3e}")


# revision 12
# speedup vs baseline: 1.1879x; 1.1879x over previous
"""Trainium2 Bass kernel for nn_BinaryDense: out = x @ (sum_k sign(b_k)*a_k) + bias.

Shapes (hardcoded): x [4096,4096] f32, b [4,4096,4096] f32, a [4,4096] f32,
bias [4096] f32 -> out [4096,4096] f32.

Tensor-parallel over the output (units) dim across 8 NeuronCores; core c owns
O-columns [c*512, (c+1)*512).

Per core: one bf16 matmul x @ w (lhsT = host-pretransposed x^T tiles
stationary, w tiles moving, fp32 PSUM) with the weight w built on-chip:
w[:, oc] = sum_k copysign(a[k,oc], b[k,:,oc]).

v2 design (from ntff trace analysis of the 264.6us baseline):
  - b tiles stream on the Scalar-engine HWDGE ring, x^T tiles on the Sync
    ring: two independent hardware DMA queues, so the startup w-build is
    never starved behind x traffic.
  - w-build on DVE is one fused scalar_tensor_tensor ((b & 0x8000) | a,
    int16 lanes) + two bf16 pair-adds for the k-sum.
  - HAM warm-up: ~11 dummy matmuls issued during the DMA/build latency so
    the PE clock-gate opens before the first real matmul; a few filler
    matmuls inside the first m-block keep it open across build-chase stalls.
  - K-blocks [4,8,20]: the kb0/kb1 partial sums are evicted by the Scalar
    engine (PSUM->SBUF copy, bf16), combined + biased on GpSimd (SBUF only),
    so DVE does nothing but builds until the single final add per m-tile
    (out = psum + acc) in kb2. No DVE evict backlog -> no PE psum stalls.
  - Last m-block runs j-outer (per-m-tile complete groups) so the final
    adds + out-DMAs of 3 of its 4 m-tiles overlap the remaining matmuls
    (short tail). Out tiles leave on the Scalar ring (idle after b).

Host side only reshapes/casts/shards (no math): x^T bf16, b -> [I,K,O] bf16,
a/bias broadcast rows (bias bf16: adds ~0.3% of one bf16 ulp vs |out|~74).
"""

import os
import sys

if "/opt/trn_rl_repo" not in sys.path:
    sys.path.insert(0, "/opt/trn_rl_repo")

import numpy as np
import ml_dtypes

BF16 = ml_dtypes.bfloat16

B = 4096   # batch rows of x
I = 4096   # input dim (contraction)
O = 4096   # output dim (sharded)
K = 4      # binary bases
NCORES = 8
OC = O // NCORES   # 512 output cols per core
P = 128

KT = I // P        # 32 k-tiles (contraction)
MT = B // P        # 32 m-tiles (output rows)
M_BLOCK = 4        # m-tiles per psum block (4 banks, x2 parity = 8)
NMB = MT // M_BLOCK


def _build_program():
    import concourse.bass as bass
    import concourse.mybir as mybir
    from concourse import bacc
    from concourse.tile import TileContext

    K_BLOCKS = [int(s) for s in os.environ.get("BK_KBLOCKS", "4,10,18").split(",")]
    assert sum(K_BLOCKS) == KT
    N_DUM = int(os.environ.get("BK_DUMMIES", "14"))
    N_FILL = int(os.environ.get("BK_FILL", "8"))
    FUSED = os.environ.get("BK_FUSED", "0") == "1"
    UPFRONT_B = int(os.environ.get("BK_UPFRONT_B", "8"))
    DB_PER_UNIT = int(os.environ.get("BK_DB_PER_UNIT", "3"))
    ADD1_GPS_EVERY = int(os.environ.get("BK_ADD1_GPS_EVERY", "0"))

    nc = bacc.Bacc(None, target_bir_lowering=False)

    b_re = nc.declare_dram_parameter("b_re", [I, K * OC], mybir.dt.bfloat16, isOutput=False)
    a_b = nc.declare_dram_parameter("a_b", [P, K * OC], mybir.dt.bfloat16, isOutput=False)
    xT = nc.declare_dram_parameter("xT", [I, B], mybir.dt.bfloat16, isOutput=False)
    bias_b = nc.declare_dram_parameter("bias_b", [P, OC], mybir.dt.bfloat16, isOutput=False)
    out = nc.declare_dram_parameter("out", [B, OC], mybir.dt.float32, isOutput=True)

    # unit schedule: one unit per (k-block, m-block)
    units = []
    k0 = 0
    for kb, KB in enumerate(K_BLOCKS):
        kts = list(range(k0, k0 + KB))
        for mb in range(NMB):
            units.append((kb, mb, kts))
        k0 += KB
    NKB = len(K_BLOCKS)

    with TileContext(nc) as tc:
        with (
            tc.tile_pool(name="const", bufs=1) as const,
            tc.tile_pool(name="bpool", bufs=7) as bpool,
            tc.tile_pool(name="cpool", bufs=3) as cpool,
            tc.tile_pool(name="tpool", bufs=3) as tpool,
            tc.tile_pool(name="wpool", bufs=1) as wpool,
            tc.tile_pool(name="xpool", bufs=24) as xpool,
            tc.tile_pool(name="accA", bufs=1) as apool,
            tc.tile_pool(name="accB", bufs=1) as apool2,
            tc.tile_pool(name="opool", bufs=4) as opool,
            tc.tile_pool(name="psum", bufs=1, space="PSUM") as psum_pool,
        ):
            # ---- consts ----
            a_tile = const.tile([P, K * OC], mybir.dt.bfloat16)
            nc.sync.dma_start(out=a_tile[:], in_=a_b[:, :])
            bias_tile = const.tile([P, OC], mybir.dt.bfloat16)
            nc.gpsimd.dma_start(out=bias_tile[:], in_=bias_b[:, :])
            mask16 = const.tile([P, 1], mybir.dt.int16)
            nc.vector.memset(mask16[:], -32768)       # 0x8000
            mask32 = const.tile([P, 1], mybir.dt.int32)
            nc.vector.memset(mask32[:], -2147450880)  # 0x80008000
            dummy_w = const.tile([P, P], mybir.dt.bfloat16)
            nc.vector.memset(dummy_w[:], 0)
            dummy_rhs = const.tile([P, OC], mybir.dt.bfloat16)
            nc.vector.memset(dummy_rhs[:], 0)

            b_tiles, w_tiles, xt_tiles = {}, {}, {}
            acc_a = {m: apool.tile([P, OC], mybir.dt.bfloat16, name=f"acc_a_{m}")
                     for m in range(MT)}
            acc_b = {m: apool2.tile([P, OC], mybir.dt.bfloat16, name=f"acc_b_{m}")
                     for m in range(MT)}

            def emit_bdma(kt):
                # NOTE: the Tile framework derives dependencies from emission
                # order — the doorbell MUST be emitted before the build that
                # reads the tile.
                bt = bpool.tile([P, K * OC], mybir.dt.bfloat16, name="b_tile")
                nc.scalar.dma_start(out=bt[:], in_=b_re[kt * P:(kt + 1) * P, :])
                b_tiles[kt] = bt

            def emit_build(kt):
                bt = b_tiles.pop(kt)
                c = cpool.tile([P, K * OC], mybir.dt.bfloat16, name="contrib")
                if FUSED:
                    nc.vector.scalar_tensor_tensor(
                        out=c.bitcast(mybir.dt.int16)[:],
                        in0=bt.bitcast(mybir.dt.int16)[:],
                        scalar=mask16[:, 0:1],
                        in1=a_tile.bitcast(mybir.dt.int16)[:],
                        op0=mybir.AluOpType.bitwise_and,
                        op1=mybir.AluOpType.bitwise_or,
                    )
                else:
                    nc.vector.tensor_scalar(
                        out=bt.bitcast(mybir.dt.int32)[:],
                        in0=bt.bitcast(mybir.dt.int32)[:],
                        scalar1=mask32[:, 0:1], scalar2=None,
                        op0=mybir.AluOpType.bitwise_and,
                    )
                    nc.vector.tensor_tensor(
                        out=c.bitcast(mybir.dt.int16)[:],
                        in0=bt.bitcast(mybir.dt.int16)[:],
                        in1=a_tile.bitcast(mybir.dt.int16)[:],
                        op=mybir.AluOpType.bitwise_or,
                    )
                t = tpool.tile([P, 2 * OC], mybir.dt.bfloat16, name="t_tile")
                # gpsimd add1-assist only for later tiles: gpsimd tt is slow
                # (~3.4us) so it must stay off the early critical chain, but
                # every assisted tile removes ~0.7us from the DVE build stream.
                add1_eng = (nc.gpsimd
                            if (ADD1_GPS_EVERY and kt >= 5
                                and kt % ADD1_GPS_EVERY == 1)
                            else nc.vector)
                add1_eng.tensor_tensor(
                    out=t[:], in0=c[:, 0:2 * OC], in1=c[:, 2 * OC:4 * OC],
                    op=mybir.AluOpType.add)
                w = wpool.tile([P, OC], mybir.dt.bfloat16, name=f"w_{kt}")
                nc.vector.tensor_tensor(
                    out=w[:], in0=t[:, 0:OC], in1=t[:, OC:2 * OC],
                    op=mybir.AluOpType.add)
                w_tiles[kt] = w

            def emit_xt(kt, mb):
                xt = xpool.tile([P, M_BLOCK * P], mybir.dt.bfloat16, name="xt")
                nc.sync.dma_start(
                    out=xt[:],
                    in_=xT[kt * P:(kt + 1) * P, mb * M_BLOCK * P:(mb + 1) * M_BLOCK * P])
                xt_tiles[(kt, mb)] = xt

            fill_parity = [0]

            def emit_filler(n):
                # alternate two banks so back-to-back dummy groups don't
                # serialize on same-bank pipeline drains
                for _ in range(n):
                    dps = psum_pool.tile([P, OC], mybir.dt.float32,
                                         name=f"ps_{4 + fill_parity[0]}")
                    fill_parity[0] ^= 1
                    nc.tensor.matmul(dps[:], dummy_w[:], dummy_rhs[:],
                                     start=True, stop=True)

            # ---- upfront: b doorbells, xt prefetch, HAM warm-up, early builds
            for kt in range(UPFRONT_B):
                emit_bdma(kt)
            for u in (0, 1):
                _, mb, kts = units[u]
                for kt in kts:
                    emit_xt(kt, mb)
            emit_filler(N_DUM)
            for kt in range(UPFRONT_B):
                emit_build(kt)
            bnext = UPFRONT_B

            # ---- main loop ----
            for u, (kb, mb, kts) in enumerate(units):
                parity = u % 2
                ps = {j: psum_pool.tile([P, OC], mybir.dt.float32,
                                        name=f"ps_{parity * 4 + j}")
                      for j in range(M_BLOCK)}
                nxt = units[u + 1] if u + 1 < len(units) else None
                last_unit = u == len(units) - 1

                if last_unit:
                    # j-outer: complete each m-tile's group, evict + store it
                    # while the next m-tile's matmuls still stream.
                    for j in range(M_BLOCK):
                        m = mb * M_BLOCK + j
                        for kt in kts:
                            nc.tensor.matmul(
                                ps[j][:],
                                xt_tiles[(kt, mb)][:, j * P:(j + 1) * P],
                                w_tiles[kt][:],
                                start=(kt == kts[0]), stop=(kt == kts[-1]))
                        o = opool.tile([P, OC], mybir.dt.float32, name="o_tile")
                        nc.vector.tensor_tensor(
                            out=o[:], in0=ps[j][:], in1=acc_a[m][:],
                            op=mybir.AluOpType.add)
                        nc.scalar.dma_start(out=out[m * P:(m + 1) * P, :], in_=o[:])
                    for kt in kts:
                        xt_tiles.pop((kt, mb))
                else:
                    done_pf = 0
                    for i, kt in enumerate(kts):
                        xt = xt_tiles.pop((kt, mb))
                        for j in range(M_BLOCK):
                            nc.tensor.matmul(
                                ps[j][:], xt[:, j * P:(j + 1) * P], w_tiles[kt][:],
                                start=(kt == kts[0]), stop=(kt == kts[-1]))
                        if u == 0 and i < len(kts) - 1:
                            emit_filler(N_FILL)
                        if u >= 1 and nxt is not None:
                            nkts = nxt[2]
                            tgt = min(((i + 1) * len(nkts) + len(kts) - 1) // len(kts),
                                      len(nkts))
                            while done_pf < tgt:
                                emit_xt(nkts[done_pf], nxt[1])
                                done_pf += 1

                    # evicts
                    for j in range(M_BLOCK):
                        m = mb * M_BLOCK + j
                        if kb == 0:
                            nc.scalar.copy(out=acc_a[m][:], in_=ps[j][:])
                        elif kb < NKB - 1:
                            nc.scalar.copy(out=acc_b[m][:], in_=ps[j][:])
                        else:
                            o = opool.tile([P, OC], mybir.dt.float32, name="o_tile")
                            nc.vector.tensor_tensor(
                                out=o[:], in0=ps[j][:], in1=acc_a[m][:],
                                op=mybir.AluOpType.add)
                            nc.scalar.dma_start(out=out[m * P:(m + 1) * P, :], in_=o[:])
                # paced b doorbells + builds (build pace must match the
                # HW ring's ~2.4us/tile, not throttle it)
                while bnext <= min(UPFRONT_B - 1 + DB_PER_UNIT * (u + 1), KT - 1):
                    emit_bdma(bnext)
                    emit_build(bnext)
                    bnext += 1

                if (not last_unit) and kb == NKB - 2:
                    # acc_a[m] += acc_b[m]; acc_a[m] += bias. bf16 SBUF adds
                    # are cheap on DVE (2x_1P, ~0.33us); emitted after this
                    # unit's build quota so they don't delay the build stream.
                    for j in range(M_BLOCK):
                        m = mb * M_BLOCK + j
                        nc.vector.tensor_tensor(
                            out=acc_a[m][:], in0=acc_a[m][:], in1=acc_b[m][:],
                            op=mybir.AluOpType.add)
                        nc.vector.tensor_tensor(
                            out=acc_a[m][:], in0=acc_a[m][:], in1=bias_tile[:],
                            op=mybir.AluOpType.add)

    nc.compile()
    return nc


_NC_CACHE = None


def _get_program():
    global _NC_CACHE
    if _NC_CACHE is None:
        _NC_CACHE = _build_program()
    return _NC_CACHE


def prep_inputs(x, b, a, bias):
    """Host-side shard/cast/layout only. Returns per-core input maps."""
    x = np.asarray(x, dtype=np.float32)
    b = np.asarray(b, dtype=np.float32)
    a = np.asarray(a, dtype=np.float32)
    bias = np.asarray(bias, dtype=np.float32)
    xT16 = np.ascontiguousarray(x.T).astype(BF16)          # [I, B] bf16
    b_iko = np.transpose(b, (1, 0, 2)).astype(BF16)        # [I, K, O] bf16
    a16 = a.astype(BF16)                                    # [K, O]
    bias16 = bias.astype(BF16)

    in_maps = []
    for c in range(NCORES):
        sl = slice(c * OC, (c + 1) * OC)
        b_slice = np.ascontiguousarray(b_iko[:, :, sl]).reshape(I, K * OC)
        a_flat = np.ascontiguousarray(a16[:, sl]).reshape(1, K * OC)
        a_bcast = np.broadcast_to(a_flat, (P, K * OC)).copy()
        bias_bcast = np.broadcast_to(bias16[sl].reshape(1, OC), (P, OC)).copy()
        in_maps.append({
            "b_re": b_slice,
            "a_b": a_bcast,
            "xT": xT16,
            "bias_b": bias_bcast,
        })
    return in_maps


def run(in_maps, trace=False):
    from concourse.bass_utils import run_bass_kernel_spmd

    nc = _get_program()
    res = run_bass_kernel_spmd(nc, in_maps, list(range(NCORES)), trace=trace)
    return res


def kernel(x, b, a, bias):
    in_maps = prep_inputs(x, b, a, bias)
    res = run(in_maps)
    out = np.concatenate([res.results[c]["out"] for c in range(NCORES)], axis=1)
    return np.ascontiguousarray(out, dtype=np.float32)


if __name__ == "__main__":
    rng = np.random.default_rng(0)
    x = rng.standard_normal((B, I), dtype=np.float32)
    b = rng.standard_normal((K, I, O), dtype=np.float32)
    a = rng.random((K, O), dtype=np.float32)
    bias = rng.standard_normal(O, dtype=np.float32)
    out = kernel(x=x, b=b, a=a, bias=bias)
    w_eff = np.einsum('kio,ko->io', np.sign(b), a.astype(np.float64)).astype(np.float64)
    expected = x.astype(np.float64) @ w_eff + bias
    rel = np.linalg.norm(out - expected) / np.linalg.norm(expected)
    print(f"rel_err = {rel:.3e}")


# revision 17
# speedup vs baseline: 1.2453x; 1.0483x over previous
"""Trainium2 Bass kernel for nn_BinaryDense: out = x @ (sum_k sign(b_k)*a_k) + bias.

Shapes (hardcoded): x [4096,4096] f32, b [4,4096,4096] f32, a [4,4096] f32,
bias [4096] f32 -> out [4096,4096] f32.

Tensor-parallel over the output (units) dim across 8 NeuronCores; core c owns
O-columns [c*512, (c+1)*512).

Per core: one bf16 matmul x @ w (lhsT = host-pretransposed x^T tiles
stationary, w tiles moving, fp32 PSUM) with the weight w built on-chip:
w[:, oc] = sum_k copysign(a[k,oc], b[k,:,oc]).

v2 design (from ntff trace analysis of the 264.6us baseline):
  - b tiles stream on the Scalar-engine HWDGE ring, x^T tiles on the Sync
    ring: two independent hardware DMA queues, so the startup w-build is
    never starved behind x traffic.
  - w-build on DVE is one fused scalar_tensor_tensor ((b & 0x8000) | a,
    int16 lanes) + two bf16 pair-adds for the k-sum.
  - HAM warm-up: ~11 dummy matmuls issued during the DMA/build latency so
    the PE clock-gate opens before the first real matmul; a few filler
    matmuls inside the first m-block keep it open across build-chase stalls.
  - K-blocks [4,8,20]: the kb0/kb1 partial sums are evicted by the Scalar
    engine (PSUM->SBUF copy, bf16), combined + biased on GpSimd (SBUF only),
    so DVE does nothing but builds until the single final add per m-tile
    (out = psum + acc) in kb2. No DVE evict backlog -> no PE psum stalls.
  - Last m-block runs j-outer (per-m-tile complete groups) so the final
    adds + out-DMAs of 3 of its 4 m-tiles overlap the remaining matmuls
    (short tail). Out tiles leave on the Scalar ring (idle after b).

Host side only reshapes/casts/shards (no math): x^T bf16, b -> [I,K,O] bf16,
a/bias broadcast rows (bias bf16: adds ~0.3% of one bf16 ulp vs |out|~74).
"""

import os
import sys

if "/opt/trn_rl_repo" not in sys.path:
    sys.path.insert(0, "/opt/trn_rl_repo")

import numpy as np
import ml_dtypes

BF16 = ml_dtypes.bfloat16

B = 4096   # batch rows of x
I = 4096   # input dim (contraction)
O = 4096   # output dim (sharded)
K = 4      # binary bases
NCORES = 8
OC = O // NCORES   # 512 output cols per core
P = 128

KT = I // P        # 32 k-tiles (contraction)
MT = B // P        # 32 m-tiles (output rows)
M_BLOCK = 4        # m-tiles per psum block (4 banks, x2 parity = 8)
NMB = MT // M_BLOCK


def _build_program():
    import concourse.bass as bass
    import concourse.mybir as mybir
    from concourse import bacc
    from concourse.tile import TileContext

    K_BLOCKS = [int(s) for s in os.environ.get("BK_KBLOCKS", "3,8,21").split(",")]
    assert sum(K_BLOCKS) == KT
    N_DUM = int(os.environ.get("BK_DUMMIES", "13"))
    N_FILL = int(os.environ.get("BK_FILL", "8"))
    FUSED = os.environ.get("BK_FUSED", "0") == "1"
    UPFRONT_B = int(os.environ.get("BK_UPFRONT_B", "8"))
    DB_PER_UNIT = int(os.environ.get("BK_DB_PER_UNIT", "3"))
    ADD1_GPS_EVERY = int(os.environ.get("BK_ADD1_GPS_EVERY", "0"))

    nc = bacc.Bacc(None, target_bir_lowering=False)

    b_re = nc.declare_dram_parameter("b_re", [I, K * OC], mybir.dt.bfloat16, isOutput=False)
    a_b = nc.declare_dram_parameter("a_b", [P, K * OC], mybir.dt.bfloat16, isOutput=False)
    xT = nc.declare_dram_parameter("xT", [I, B], mybir.dt.bfloat16, isOutput=False)
    bias_b = nc.declare_dram_parameter("bias_b", [P, OC], mybir.dt.bfloat16, isOutput=False)
    out = nc.declare_dram_parameter("out", [B, OC], mybir.dt.float32, isOutput=True)

    # unit schedule: one unit per (k-block, m-block)
    units = []
    k0 = 0
    for kb, KB in enumerate(K_BLOCKS):
        kts = list(range(k0, k0 + KB))
        for mb in range(NMB):
            units.append((kb, mb, kts))
        k0 += KB
    NKB = len(K_BLOCKS)

    with TileContext(nc) as tc:
        with (
            tc.tile_pool(name="const", bufs=1) as const,
            tc.tile_pool(name="bpool", bufs=8) as bpool,
            tc.tile_pool(name="cpool", bufs=3) as cpool,
            tc.tile_pool(name="tpool", bufs=3) as tpool,
            tc.tile_pool(name="wpool", bufs=1) as wpool,
            tc.tile_pool(name="xpool", bufs=24) as xpool,
            tc.tile_pool(name="accA", bufs=1) as apool,
            tc.tile_pool(name="accB", bufs=1) as apool2,
            tc.tile_pool(name="opool", bufs=4) as opool,
            tc.tile_pool(name="psum", bufs=1, space="PSUM") as psum_pool,
        ):
            # ---- consts ----
            a_tile = const.tile([P, K * OC], mybir.dt.bfloat16)
            nc.sync.dma_start(out=a_tile[:], in_=a_b[:, :])
            bias_tile = const.tile([P, OC], mybir.dt.bfloat16)
            nc.gpsimd.dma_start(out=bias_tile[:], in_=bias_b[:, :])
            mask16 = const.tile([P, 1], mybir.dt.int16)
            nc.vector.memset(mask16[:], -32768)       # 0x8000
            mask32 = const.tile([P, 1], mybir.dt.int32)
            nc.vector.memset(mask32[:], -2147450880)  # 0x80008000
            dummy_w = const.tile([P, P], mybir.dt.bfloat16)
            nc.vector.memset(dummy_w[:], 0)
            dummy_rhs = const.tile([P, OC], mybir.dt.bfloat16)
            nc.vector.memset(dummy_rhs[:], 0)

            b_tiles, w_tiles, xt_tiles = {}, {}, {}
            acc_a = {m: apool.tile([P, OC], mybir.dt.bfloat16, name=f"acc_a_{m}")
                     for m in range(MT)}
            acc_b = {m: apool2.tile([P, OC], mybir.dt.bfloat16, name=f"acc_b_{m}")
                     for m in range(MT)}

            def emit_bdma(kt, halves=False):
                # NOTE: the Tile framework derives dependencies from emission
                # order — the doorbell MUST be emitted before the build that
                # reads the tile.
                bt = bpool.tile([P, K * OC], mybir.dt.bfloat16, name="b_tile")
                if halves:
                    # two half-transfers so the first half (k0,k1) lands
                    # ~2.3us earlier and the build chain starts sooner
                    h = K * OC // 2
                    nc.scalar.dma_start(out=bt[:, 0:h],
                                        in_=b_re[kt * P:(kt + 1) * P, 0:h])
                    nc.scalar.dma_start(out=bt[:, h:],
                                        in_=b_re[kt * P:(kt + 1) * P, h:])
                else:
                    nc.scalar.dma_start(out=bt[:], in_=b_re[kt * P:(kt + 1) * P, :])
                b_tiles[kt] = bt

            def emit_build0_halves():
                # kt=0 critical path: process each half as it lands.
                # halves are k-major: h0=[c0|c1], h1=[c2|c3]; pair within
                # halves instead of across (same sum).
                bt = b_tiles.pop(0)
                c = cpool.tile([P, K * OC], mybir.dt.bfloat16, name="contrib")
                t = tpool.tile([P, 2 * OC], mybir.dt.bfloat16, name="t_tile")
                for h in range(2):
                    lo32, hi32 = h * OC, (h + 1) * OC          # int32 view cols
                    lo16, hi16 = h * 2 * OC, (h + 1) * 2 * OC  # int16 view cols
                    nc.vector.tensor_scalar(
                        out=bt.bitcast(mybir.dt.int32)[:, lo32:hi32],
                        in0=bt.bitcast(mybir.dt.int32)[:, lo32:hi32],
                        scalar1=mask32[:, 0:1], scalar2=None,
                        op0=mybir.AluOpType.bitwise_and)
                    nc.vector.tensor_tensor(
                        out=c.bitcast(mybir.dt.int16)[:, lo16:hi16],
                        in0=bt.bitcast(mybir.dt.int16)[:, lo16:hi16],
                        in1=a_tile.bitcast(mybir.dt.int16)[:, lo16:hi16],
                        op=mybir.AluOpType.bitwise_or)
                    nc.vector.tensor_tensor(
                        out=t[:, h * OC:(h + 1) * OC],
                        in0=c[:, h * 2 * OC:h * 2 * OC + OC],
                        in1=c[:, h * 2 * OC + OC:(h + 1) * 2 * OC],
                        op=mybir.AluOpType.add)
                w = wpool.tile([P, OC], mybir.dt.bfloat16, name="w_0")
                nc.vector.tensor_tensor(
                    out=w[:], in0=t[:, 0:OC], in1=t[:, OC:2 * OC],
                    op=mybir.AluOpType.add)
                w_tiles[0] = w

            def emit_build(kt):
                bt = b_tiles.pop(kt)
                c = cpool.tile([P, K * OC], mybir.dt.bfloat16, name="contrib")
                if FUSED:
                    nc.vector.scalar_tensor_tensor(
                        out=c.bitcast(mybir.dt.int16)[:],
                        in0=bt.bitcast(mybir.dt.int16)[:],
                        scalar=mask16[:, 0:1],
                        in1=a_tile.bitcast(mybir.dt.int16)[:],
                        op0=mybir.AluOpType.bitwise_and,
                        op1=mybir.AluOpType.bitwise_or,
                    )
                else:
                    nc.vector.tensor_scalar(
                        out=bt.bitcast(mybir.dt.int32)[:],
                        in0=bt.bitcast(mybir.dt.int32)[:],
                        scalar1=mask32[:, 0:1], scalar2=None,
                        op0=mybir.AluOpType.bitwise_and,
                    )
                    nc.vector.tensor_tensor(
                        out=c.bitcast(mybir.dt.int16)[:],
                        in0=bt.bitcast(mybir.dt.int16)[:],
                        in1=a_tile.bitcast(mybir.dt.int16)[:],
                        op=mybir.AluOpType.bitwise_or,
                    )
                t = tpool.tile([P, 2 * OC], mybir.dt.bfloat16, name="t_tile")
                # gpsimd add1-assist only for later tiles: gpsimd tt is slow
                # (~3.4us) so it must stay off the early critical chain, but
                # every assisted tile removes ~0.7us from the DVE build stream.
                add1_eng = (nc.gpsimd
                            if (ADD1_GPS_EVERY and kt >= 5
                                and kt % ADD1_GPS_EVERY == 1)
                            else nc.vector)
                add1_eng.tensor_tensor(
                    out=t[:], in0=c[:, 0:2 * OC], in1=c[:, 2 * OC:4 * OC],
                    op=mybir.AluOpType.add)
                w = wpool.tile([P, OC], mybir.dt.bfloat16, name=f"w_{kt}")
                nc.vector.tensor_tensor(
                    out=w[:], in0=t[:, 0:OC], in1=t[:, OC:2 * OC],
                    op=mybir.AluOpType.add)
                w_tiles[kt] = w

            def emit_xt(kt, mb):
                xt = xpool.tile([P, M_BLOCK * P], mybir.dt.bfloat16, name="xt")
                nc.sync.dma_start(
                    out=xt[:],
                    in_=xT[kt * P:(kt + 1) * P, mb * M_BLOCK * P:(mb + 1) * M_BLOCK * P])
                xt_tiles[(kt, mb)] = xt

            fill_parity = [0]

            def emit_filler(n):
                # alternate two banks so back-to-back dummy groups don't
                # serialize on same-bank pipeline drains
                for _ in range(n):
                    dps = psum_pool.tile([P, OC], mybir.dt.float32,
                                         name=f"ps_{4 + fill_parity[0]}")
                    fill_parity[0] ^= 1
                    nc.tensor.matmul(dps[:], dummy_w[:], dummy_rhs[:],
                                     start=True, stop=True)

            # ---- emission-time PE clock model (ns). Drives filler sizing and
            # doorbell pacing; only needs ~15% accuracy.
            MM_NS = 216.0
            EST_W0 = float(os.environ.get("BK_EST_W0", "13200"))
            EST_BUILD = float(os.environ.get("BK_EST_BUILD", "3350"))

            def build_done(kt):
                return EST_W0 + EST_BUILD * kt

            # ---- upfront: b doorbells, xt prefetch, HAM warm-up, early builds
            emit_bdma(0, halves=True)
            for kt in range(1, UPFRONT_B):
                emit_bdma(kt)
            for u in (0, 1):
                _, mb, kts = units[u]
                for kt in kts:
                    emit_xt(kt, mb)
            emit_filler(N_DUM)
            emit_build0_halves()
            for kt in range(1, UPFRONT_B):
                emit_build(kt)
            bnext_ref = [UPFRONT_B]
            pe_t = build_done(0)
            used = set()
            pending_combines = []

            def pace(cur_t):
                # emit b doorbells+builds when their bpool slot is (about to
                # be) free; once all builds are emitted, flush the deferred
                # acc combines so they sit after every build in the DVE FIFO.
                while (bnext_ref[0] <= KT - 1
                       and cur_t >= build_done(bnext_ref[0] - 8) - 1100):
                    emit_bdma(bnext_ref[0])
                    emit_build(bnext_ref[0])
                    bnext_ref[0] += 1
                if bnext_ref[0] == KT and pending_combines:
                    for m in list(pending_combines):
                        nc.vector.tensor_tensor(
                            out=acc_a[m][:], in0=acc_a[m][:], in1=acc_b[m][:],
                            op=mybir.AluOpType.add)
                        nc.vector.tensor_tensor(
                            out=acc_a[m][:], in0=acc_a[m][:], in1=bias_tile[:],
                            op=mybir.AluOpType.add)
                    pending_combines.clear()

            # ---- main loop ----
            for u, (kb, mb, kts) in enumerate(units):
                parity = u % 2
                ps = {j: psum_pool.tile([P, OC], mybir.dt.float32,
                                        name=f"ps_{parity * 4 + j}")
                      for j in range(M_BLOCK)}
                nxt = units[u + 1] if u + 1 < len(units) else None
                last_unit = u == len(units) - 1

                if last_unit:
                    # j-outer: complete each m-tile's group, evict + store it
                    # while the next m-tile's matmuls still stream.
                    for j in range(M_BLOCK):
                        m = mb * M_BLOCK + j
                        for kt in kts:
                            nc.tensor.matmul(
                                ps[j][:],
                                xt_tiles[(kt, mb)][:, j * P:(j + 1) * P],
                                w_tiles[kt][:],
                                start=(kt == kts[0]), stop=(kt == kts[-1]))
                        pe_t += len(kts) * MM_NS
                        o = opool.tile([P, OC], mybir.dt.float32, name="o_tile")
                        nc.vector.tensor_tensor(
                            out=o[:], in0=ps[j][:], in1=acc_a[m][:],
                            op=mybir.AluOpType.add)
                        nc.scalar.dma_start(out=out[m * P:(m + 1) * P, :], in_=o[:])
                    for kt in kts:
                        xt_tiles.pop((kt, mb))
                else:
                    done_pf = 0
                    for i, kt in enumerate(kts):
                        if kt not in used:
                            used.add(kt)
                            stall = build_done(kt) - pe_t
                            if stall > 350:
                                nfill = min(int(stall * 0.8 / MM_NS), 25)
                                emit_filler(nfill)
                                pe_t += nfill * MM_NS
                            pe_t = max(pe_t, build_done(kt))
                        xt = xt_tiles.pop((kt, mb))
                        for j in range(M_BLOCK):
                            nc.tensor.matmul(
                                ps[j][:], xt[:, j * P:(j + 1) * P], w_tiles[kt][:],
                                start=(kt == kts[0]), stop=(kt == kts[-1]))
                        pe_t += M_BLOCK * MM_NS
                        pace(pe_t)
                        if u >= 1 and nxt is not None:
                            nkts = nxt[2]
                            tgt = min(((i + 1) * len(nkts) + len(kts) - 1) // len(kts),
                                      len(nkts))
                            while done_pf < tgt:
                                emit_xt(nkts[done_pf], nxt[1])
                                done_pf += 1

                    # evicts
                    for j in range(M_BLOCK):
                        m = mb * M_BLOCK + j
                        if kb == 0:
                            nc.scalar.copy(out=acc_a[m][:], in_=ps[j][:])
                        elif kb < NKB - 1:
                            nc.scalar.copy(out=acc_b[m][:], in_=ps[j][:])
                            pending_combines.append(m)
                        else:
                            o = opool.tile([P, OC], mybir.dt.float32, name="o_tile")
                            nc.vector.tensor_tensor(
                                out=o[:], in0=ps[j][:], in1=acc_a[m][:],
                                op=mybir.AluOpType.add)
                            nc.scalar.dma_start(out=out[m * P:(m + 1) * P, :], in_=o[:])
                    pace(pe_t)

    nc.compile()
    return nc


_NC_CACHE = None


def _get_program():
    global _NC_CACHE
    if _NC_CACHE is None:
        _NC_CACHE = _build_program()
    return _NC_CACHE


def prep_inputs(x, b, a, bias):
    """Host-side shard/cast/layout only. Returns per-core input maps."""
    x = np.asarray(x, dtype=np.float32)
    b = np.asarray(b, dtype=np.float32)
    a = np.asarray(a, dtype=np.float32)
    bias = np.asarray(bias, dtype=np.float32)
    xT16 = np.ascontiguousarray(x.T).astype(BF16)          # [I, B] bf16
    b_iko = np.transpose(b, (1, 0, 2)).astype(BF16)        # [I, K, O] bf16
    a16 = a.astype(BF16)                                    # [K, O]
    bias16 = bias.astype(BF16)

    in_maps = []
    for c in range(NCORES):
        sl = slice(c * OC, (c + 1) * OC)
        b_slice = np.ascontiguousarray(b_iko[:, :, sl]).reshape(I, K * OC)
        a_flat = np.ascontiguousarray(a16[:, sl]).reshape(1, K * OC)
        a_bcast = np.broadcast_to(a_flat, (P, K * OC)).copy()
        bias_bcast = np.broadcast_to(bias16[sl].reshape(1, OC), (P, OC)).copy()
        in_maps.append({
            "b_re": b_slice,
            "a_b": a_bcast,
            "xT": xT16,
            "bias_b": bias_bcast,
        })
    return in_maps


def run(in_maps, trace=False):
    from concourse.bass_utils import run_bass_kernel_spmd

    nc = _get_program()
    res = run_bass_kernel_spmd(nc, in_maps, list(range(NCORES)), trace=trace)
    return res


def kernel(x, b, a, bias):
    in_maps = prep_inputs(x, b, a, bias)
    res = run(in_maps)
    out = np.concatenate([res.results[c]["out"] for c in range(NCORES)], axis=1)
    return np.ascontiguousarray(out, dtype=np.float32)


if __name__ == "__main__":
    rng = np.random.default_rng(0)
    x = rng.standard_normal((B, I), dtype=np.float32)
    b = rng.standard_normal((K, I, O), dtype=np.float32)
    a = rng.random((K, O), dtype=np.float32)
    bias = rng.standard_normal(O, dtype=np.float32)
    out = kernel(x=x, b=b, a=a, bias=bias)
    w_eff = np.einsum('kio,ko->io', np.sign(b), a.astype(np.float64)).astype(np.float64)
    expected = x.astype(np.float64) @ w_eff + bias
    rel = np.linalg.norm(out - expected) / np.linalg.norm(expected)
    print(f"rel_err = {rel:.3e}")
